# revision 3
# baseline (speedup 1.0000x reference)
"""RNN-T JointNetwork kernel for Trainium2 (Bass/Tile), 8 NeuronCores.

Problem: out = log_softmax(enc@W1 [T,V] (+) dec@W2+b [U,V], axis=V)
  shapes: B=8, T=256, U=64, H=256 (2H=512), V=1024; out [B,T,U,V] fp32.

Strategy: data-parallel over batch (1 batch element per core). The output
(64 MB/core, 512 MB total) dominates -> memory-bound; all compute is
hidden under the output DMA.

Key tricks:
  - logsumexp over V without touching the [T,U,V] tensor:
      sum_v exp(e[t,v]+d[u,v]) = exp(e[t])·exp(d[u])  (a [V]-contraction
      matmul), so S=[T,U] comes from a tiny PE matmul of exp(enc_projT)
      against exp(dec_projT).
  - each output tile [128t, 1024v] is built in PSUM by the PE:
      ones[1,128] (x) dec_row (broadcast) + I[128] @ enc_proj (add),
    using float32r (1 cyc/row; ~1.2e-4 rounding) instead of fp32
    (4 cyc/row).
  - single fused elementwise pass moves PSUM->SBUF while adding
    -log S[t,u] (per-partition scalar), alternating ScalarE / VectorE.
"""

import numpy as np

import concourse.bass as bass
import concourse.bacc as bacc
import concourse.tile as tile
from concourse import mybir
from concourse.bass_utils import run_bass_kernel_spmd
from concourse.masks import make_identity

F32 = mybir.dt.float32
F32R = mybir.dt.float32r
AF = mybir.ActivationFunctionType

B, T, U, H, V = 8, 256, 64, 256, 1024
UG = 8          # u's per output buffer / staging group
N_CORES = 8


def build_nc(reps=1, out_dma="gpsimd"):
    nc = bacc.Bacc("TRN2", target_bir_lowering=False, debug=False,
                   num_devices=N_CORES)

    enc = nc.dram_tensor("enc", [T, H], F32, kind="ExternalInput")
    dec = nc.dram_tensor("dec", [U, H], F32, kind="ExternalInput")
    W = nc.dram_tensor("W", [2 * H, V], F32, kind="ExternalInput")
    bias = nc.dram_tensor("b", [V], F32, kind="ExternalInput")
    out = nc.dram_tensor("out", [T, U, V], F32, kind="ExternalOutput")
    tick = nc.dram_tensor("tick", [1, 1], F32, kind="ExternalOutput")

    # DRAM scratch holding dec_proj+b rows pre-rounded to f32r, so the main
    # loop can pull [1, UG*V] staging rows (contiguous in DRAM) to
    # partition 0 for the PE broadcast matmuls.
    decb_dram = nc.dram_tensor("decb_scratch", [U, V], F32R)

    with tile.TileContext(nc) as tc:
        with tc.tile_pool(name="singles", bufs=1) as singles:
            # ---------------- constants & inputs ----------------
            ident_f = singles.tile([128, 128], F32)
            make_identity(nc, ident_f)
            ident_r = singles.tile([128, 128], F32R)
            nc.vector.tensor_copy(ident_r, ident_f)
            ones_f = singles.tile([1, 128], F32)
            nc.vector.memset(ones_f, 1.0)
            ones_r = singles.tile([1, 128], F32R)
            nc.vector.tensor_copy(ones_r, ones_f)

            enc_sb = singles.tile([128, 2, H], F32)   # [t_local, t_blk, h]
            nc.gpsimd.dma_start(out=enc_sb,
                                in_=enc.rearrange("(i p) h -> p i h", p=128))
            dec_sb = singles.tile([64, H], F32)
            nc.gpsimd.dma_start(out=dec_sb, in_=dec[:, :])
            w_sb = singles.tile([128, 4, V], F32)      # chunk c = W rows 128c..
            nc.gpsimd.dma_start(out=w_sb,
                                in_=W.rearrange("(c p) v -> p c v", p=128))
            b_colt = singles.tile([128, 8], F32)       # b in vblk-column form
            nc.gpsimd.dma_start(out=b_colt,
                                in_=bias.rearrange("(c p) -> p c", p=128))
            b_row = singles.tile([1, V], F32)
            nc.gpsimd.dma_start(out=b_row,
                                in_=bias.rearrange("(o v) -> o v", o=1))

            # persistent intermediates
            encT = singles.tile([128, 2, T], F32)      # [h_local, h_chunk, t]
            decT = singles.tile([128, 2, 64], F32)     # [h_local, h_chunk, u]
            exp_encT = singles.tile([128, 8, T], F32)  # [v_local, v_blk, t]
            exp_decT = singles.tile([128, 8, 64], F32)
            encp_r = singles.tile([128, 2, V], F32R)   # enc_proj rows (f32r)
            decb_r = singles.tile([64, V], F32R)       # dec_proj+b (f32r)
            neglogs = singles.tile([128, 2, 64], F32)  # -log S  [t_loc, tb, u]

            # ---------------- prologue ----------------
            with tc.tile_pool(name="pro_psum", bufs=3, space="PSUM") as pp:
                # transposes: encT, decT via PE
                for i in range(2):          # t block
                    for c in range(2):      # h chunk
                        pt = pp.tile([128, 1024], F32, tag="pp")
                        nc.tensor.transpose(pt[:, 0:128],
                                            enc_sb[:, i, 128 * c:128 * (c + 1)],
                                            ident_f)
                        nc.vector.tensor_copy(
                            encT[:, c, 128 * i:128 * (i + 1)], pt[:, 0:128])
                for c in range(2):
                    pt = pp.tile([128, 1024], F32, tag="pp")
                    nc.tensor.transpose(pt[:, 0:64],
                                        dec_sb[:, 128 * c:128 * (c + 1)],
                                        ident_f[0:64, 0:64])
                    nc.vector.tensor_copy(decT[:, c, :], pt[:, 0:64])

                # enc_projT -> exp ; dec_projT + b -> exp   (fp32 matmuls)
                for vb in range(8):
                    pt = pp.tile([128, 1024], F32, tag="pp")
                    for c in range(2):
                        nc.tensor.matmul(
                            pt[:, 0:T],
                            w_sb[:, c, 128 * vb:128 * (vb + 1)],
                            encT[:, c, :],
                            start=(c == 0), stop=(c == 1))
                    nc.scalar.activation(exp_encT[:, vb, :], pt[:, 0:T], AF.Exp)
                for vb in range(8):
                    pt = pp.tile([128, 1024], F32, tag="pp")
                    for c in range(2):
                        nc.tensor.matmul(
                            pt[:, 0:64],
                            w_sb[:, 2 + c, 128 * vb:128 * (vb + 1)],
                            decT[:, c, :],
                            start=(c == 0), stop=(c == 1))
                    nc.scalar.activation(exp_decT[:, vb, :], pt[:, 0:64],
                                         AF.Exp, bias=b_colt[:, vb:vb + 1])

                # enc_proj (non-T) -> f32r
                for tb in range(2):
                    pt = pp.tile([128, 1024], F32, tag="pp")
                    for c in range(2):
                        for nh in range(2):
                            nc.tensor.matmul(
                                pt[:, 512 * nh:512 * (nh + 1)],
                                encT[:, c, 128 * tb:128 * (tb + 1)],
                                w_sb[:, c, 512 * nh:512 * (nh + 1)],
                                start=(c == 0), stop=(c == 1))
                    nc.vector.tensor_copy(encp_r[:, tb, :], pt)

                # dec_proj + b (non-T) -> f32r -> DRAM scratch
                pt_d = pp.tile([128, 1024], F32, tag="pp")
                for nh in range(2):
                    for c in range(2):
                        nc.tensor.matmul(
                            pt_d[0:64, 512 * nh:512 * (nh + 1)],
                            decT[:, c, :],
                            w_sb[:, 2 + c, 512 * nh:512 * (nh + 1)],
                            start=(c == 0), stop=False)
                    nc.tensor.matmul(
                        pt_d[0:64, 512 * nh:512 * (nh + 1)],
                        ones_f[0:1, 0:64],
                        b_row[0:1, 512 * nh:512 * (nh + 1)],
                        start=False, stop=True)
                nc.vector.tensor_copy(decb_r, pt_d[0:64, :])
                nc.gpsimd.dma_start(out=decb_dram[:, :], in_=decb_r)

                # S[t,u] = sum_v exp_encT[v,t]*exp_decT[v,u]; neglogs = -ln S
                for tb in range(2):
                    pt = pp.tile([128, 1024], F32, tag="pp")
                    for vb in range(8):
                        nc.tensor.matmul(
                            pt[:, 0:64],
                            exp_encT[:, vb, 128 * tb:128 * (tb + 1)],
                            exp_decT[:, vb, :],
                            start=(vb == 0), stop=(vb == 7))
                    rec = singles.tile([128, 64], F32, tag=f"rec{tb}")
                    nc.vector.reciprocal(rec, pt[:, 0:64])
                    nc.scalar.activation(neglogs[:, tb, :], rec, AF.Ln)

            # ---------------- main loop ----------------
            with tc.tile_pool(name="mp", bufs=4, space="PSUM") as mp, \
                 tc.tile_pool(name="ob", bufs=2) as obp, \
                 tc.tile_pool(name="st", bufs=2) as stp:
                decb_rows = decb_dram.rearrange("(a g) v -> a (g v)", g=UG)
                out_dma_start = (nc.sync.dma_start if out_dma == "sync"
                                 else nc.gpsimd.dma_start)

                def main_body():
                    for tb in range(2):
                        for ug in range(64 // UG):
                            stage = stp.tile([1, UG * V], F32R, tag="st")
                            nc.gpsimd.dma_start(out=stage,
                                                in_=decb_rows[ug:ug + 1, :])
                            ob = obp.tile([128, UG * V], F32, tag="ob")
                            for uu in range(UG):
                                u = UG * ug + uu
                                ps = mp.tile([128, V], F32, tag="mp")
                                on_dve = (uu % 2 == 1)
                                for nh in range(2):
                                    nc.tensor.matmul(
                                        ps[:, 512 * nh:512 * (nh + 1)],
                                        ones_r,
                                        stage[0:1, V * uu + 512 * nh:
                                              V * uu + 512 * (nh + 1)],
                                        start=True, stop=on_dve)
                                    if not on_dve:
                                        nc.tensor.matmul(
                                            ps[:, 512 * nh:512 * (nh + 1)],
                                            ident_r,
                                            encp_r[:, tb,
                                                   512 * nh:512 * (nh + 1)],
                                            start=False, stop=True)
                                obs = ob[:, V * uu:V * (uu + 1)]
                                nb = neglogs[:, tb, u:u + 1]
                                if on_dve:
                                    nc.vector.scalar_tensor_tensor(
                                        out=obs, in0=ps, scalar=nb,
                                        in1=encp_r[:, tb, :].bitcast(F32),
                                        op0=mybir.AluOpType.add,
                                        op1=mybir.AluOpType.add)
                                else:
                                    nc.scalar.add(obs, ps, nb)
                            out_dma_start(
                                out=out[128 * tb:128 * (tb + 1),
                                        UG * ug:UG * (ug + 1), :],
                                in_=ob.rearrange("p (g v) -> p g v", g=UG))

                if reps == 1:
                    main_body()
                else:
                    with tc.For_i(0, reps, 1):
                        main_body()

                tk = stp.tile([1, 1], F32, tag="tk")
                nc.vector.memset(tk, 1.0)
                nc.gpsimd.dma_start(out=tick[:, :], in_=tk)
    nc.compile()
    return nc


_NC = None


def _get_nc():
    global _NC
    if _NC is None:
        _NC = build_nc()
    return _NC


def kernel(encoder_outputs, decoder_outputs, W, b):
    nc = _get_nc()
    enc = np.ascontiguousarray(np.asarray(encoder_outputs, dtype=np.float32))
    dec = np.ascontiguousarray(np.asarray(decoder_outputs, dtype=np.float32))
    Wf = np.ascontiguousarray(np.asarray(W, dtype=np.float32))
    bf = np.ascontiguousarray(np.asarray(b, dtype=np.float32))
    in_maps = [
        {"enc": enc[i], "dec": dec[i], "W": Wf, "b": bf}
        for i in range(N_CORES)
    ]
    res = run_bass_kernel_spmd(nc, in_maps, core_ids=list(range(N_CORES)))
    return np.stack([res.results[i]["out"] for i in range(N_CORES)], axis=0)


if __name__ == "__main__":
    rng = np.random.default_rng(0)
    outs = kernel(
        encoder_outputs=rng.standard_normal((B, T, H)).astype(np.float32),
        decoder_outputs=rng.standard_normal((B, U, H)).astype(np.float32),
        W=(rng.standard_normal((2 * H, V)) / np.sqrt(2 * H)).astype(np.float32),
        b=np.zeros(V, np.float32),
    )
    print(outs.shape, outs.dtype)
